# revision 23
# baseline (speedup 1.0000x reference)
"""CustomLSTMCell fused kernel for 8x Trainium2 NeuronCores.

Reference computation (B=8192, D=H=1024):
    z = e_t @ W_x.T + h_prev @ W_h.T + (b_x + b_h + b_extra)   # [B, 4H]
    f, i, o, c = split(z, 4)
    c_t = sigmoid(f) * c_prev + sigmoid(i) * tanh(c)
    h_t = sigmoid(o) * tanh(c_t)

Sharding: 2-way batch x 4-way hidden-unit (8 cores, no collectives).
Each core computes z transposed ([gate_rows, batch] layout) so the bias
folds into the ScalarE activation's per-partition bias operand, and both
matmul operands arrive pre-transposed from the host (contraction dim on
partitions).  Matmuls run in float32r (full-rate fp32 path on TRN2 for
moving free-dim >= 256, ~2^-13 effective mantissa).

Schedule notes (measured on HW, ~267us total; PE matmul floor is ~240us):
 - W and the first batch-block's x are DMAed as interleaved per-k-chunk
   tiles on the sync HWDGE ring; the first batch block runs its matmuls
   k-outer so the PE starts as soon as chunk 0 lands (~13us in) and
   chases the arrival stream instead of waiting ~43us for the full 12MB.
 - Each DMA_DIRECT2D costs ~650ns of issue time on its engine, and a DMA
   chain on the scalar engine develops issue-blocking sem waits that
   delay ACTIVATEs (which gate PSUM-bank release).  So: bulk input
   stream on sync, cprev + steady-state outputs on gpsimd (SWDGE), and
   the scalar ring carries only the bias load and the final outputs.
 - The final quadruple's elementwise runs in two half-width chunks to
   shorten the post-matmul tail; the Tile epilogue (drain + sem-reset
   butterfly) is ~10us of fixed cost after that.
"""

import sys

if "/opt/trn_rl_repo" not in sys.path:
    sys.path.insert(0, "/opt/trn_rl_repo")

import numpy as np

import concourse.bass as bass
import concourse.mybir as mybir
from concourse import bacc
from concourse.bass_utils import run_bass_kernel_spmd
from concourse.tile import TileContext

F32 = mybir.dt.float32
F32R = mybir.dt.float32r
AFT = mybir.ActivationFunctionType
ALU = mybir.AluOpType

B, D, H = 8192, 1024, 1024
M_BATCH, M_UNIT = 2, 4          # batch split x unit split = 8 cores
BS = B // M_BATCH               # 4096 batch rows per core
U = H // M_UNIT                 # 256 hidden units per core
K = D + H                       # 2048 contraction (e_t | h_prev)
KT = K // 128                   # 16 k-chunks
G = 4 * U                       # 1024 gate rows per core (f|i|o|c x U)
BBLK = 512                      # moving free-dim per matmul
NBB = BS // BBLK                # 8 batch blocks
NJ = U // 128                   # 2 unit sub-blocks of 128 partitions

GATE_FUNCS = [AFT.Sigmoid, AFT.Sigmoid, AFT.Sigmoid, AFT.Tanh]  # f, i, o, c


def _build_nc():
    nc = bacc.Bacc()

    xT = nc.dram_tensor("xT", [K, BS], F32R, kind="ExternalInput")
    wT = nc.dram_tensor("wT", [K, G], F32R, kind="ExternalInput")
    bias = nc.dram_tensor("bias", [G], F32, kind="ExternalInput")
    cT = nc.dram_tensor("cT", [U, BS], F32, kind="ExternalInput")
    hT_out = nc.dram_tensor("hT_out", [U, BS], F32, kind="ExternalOutput")
    cT_out = nc.dram_tensor("cT_out", [U, BS], F32, kind="ExternalOutput")

    xT_r = xT.ap().rearrange("(k p) b -> p k b", p=128)      # [128, KT, BS]
    wT_r = wT.ap().rearrange("(k p) g -> p k g", p=128)      # [128, KT, G]
    bias_r = bias.ap().rearrange("(c p) -> p c", p=128)      # [128, 4*NJ]
    cT_r = cT.ap().rearrange("(j p) b -> p j b", p=128)      # [128, NJ, BS]
    hT_r = hT_out.ap().rearrange("(j p) b -> p j b", p=128)
    cTo_r = cT_out.ap().rearrange("(j p) b -> p j b", p=128)

    with TileContext(nc) as tc:
        with (
            tc.tile_pool(name="wpool", bufs=1) as wpool,
            tc.tile_pool(name="xpool", bufs=2) as xpool,
            tc.tile_pool(name="cpool", bufs=2) as cpool,
            tc.tile_pool(name="gpool", bufs=2) as gpool,
            tc.tile_pool(name="opool", bufs=2) as opool,
            tc.tile_pool(name="psum", bufs=2, space="PSUM") as pp,
        ):
            # Per-k W tiles, DMA-interleaved with the first batch block's x
            # tiles so the PE can start on chunk 0 immediately and chase the
            # arrival stream.  All on the sync ring: its ~650ns/DMA issue
            # rate paces arrivals in k-order, and the scalar engine must stay
            # free for ACTIVATEs (a DMA chain there develops issue-blocking
            # sem waits that stall PSUM release by ~8us).
            bias_sb = wpool.tile([128, 4 * NJ], F32)
            nc.scalar.dma_start(out=bias_sb[:], in_=bias_r)

            # PE warm-up: ~12 throwaway matmuls on zeroed scratch while the
            # first W/x chunks are still in flight.  The HAM clock gate needs
            # ~3.4us of sustained PE activity to lift the PE from 1.2GHz to
            # 2.4GHz; without this the first ~13 real matmuls run cold.
            warm = wpool.tile([128, BBLK], F32, name="warm")
            nc.vector.memset(warm[:], 0.0)
            warm_r = warm.bitcast(F32R)
            warm_ps = pp.tile([128, BBLK], F32, tag="ps0", name="warm_ps")
            # 10 cold-rate matmuls ~= 4.3us: enough sustained activity to trip
            # the HAM SHORT window, finishing just before the first W/x
            # chunks land (~13us) so they never delay real work.
            for _ in range(10):
                nc.tensor.matmul(
                    warm_ps[:], warm_r[:, 0:128], warm_r[:], start=True, stop=True
                )

            w_sb = []
            x0_sb = []
            for k in range(KT):
                wt = wpool.tile([128, G], F32R, tag=f"w{k}", name=f"w{k}")
                nc.sync.dma_start(out=wt[:], in_=wT_r[:, k, :])
                w_sb.append(wt)
                if k % 2 == 0:
                    xp = xpool.tile(
                        [128, 2, BBLK], F32R, tag=f"xp{k // 2}", name=f"xp{k // 2}", bufs=3
                    )
                nc.sync.dma_start(out=xp[:, k % 2, :], in_=xT_r[:, k, 0:BBLK])
                x0_sb.append(xp[:, k % 2, :])

            def load_cprev(bb):
                t = cpool.tile([128, NJ, BBLK], F32, tag="cprev", name="cprev")
                nc.gpsimd.dma_start(
                    out=t[:], in_=cT_r[:, :, bb * BBLK:(bb + 1) * BBLK]
                )
                return t

            def elementwise(ps, cprev_sb, bb, j, n_split=1, out_ring=None):
                """Gate nonlinearities + cell update for one quadruple."""
                out_ring = out_ring or nc.gpsimd
                w = BBLK // n_split
                for s in range(n_split):
                    csl = slice(s * w, (s + 1) * w)

                    def gate(g):
                        at = gpool.tile([128, w], F32, tag=f"act{g}", name=f"act{g}")
                        nc.scalar.activation(
                            at[:], ps[g][:, csl], GATE_FUNCS[g],
                            bias=bias_sb[:, 2 * g + j: 2 * g + j + 1],
                        )
                        return at

                    # ACT stream order mirrors the (c,f,i,o) matmul order so
                    # the c_t chain completes before the o-gate's matmuls do.
                    mc = gate(3)
                    gf = gate(0)
                    gi = gate(1)
                    t1 = gpool.tile([128, w], F32, tag="t1", name="t1")
                    nc.vector.tensor_tensor(
                        t1[:], gf[:], cprev_sb[:, j, csl], ALU.mult
                    )
                    t2 = gpool.tile([128, w], F32, tag="t2", name="t2")
                    nc.vector.tensor_tensor(t2[:], gi[:], mc[:], ALU.mult)
                    ct = opool.tile([128, w], F32, tag="ct", name="ct")
                    nc.vector.tensor_tensor(ct[:], t1[:], t2[:], ALU.add)
                    th = gpool.tile([128, w], F32, tag="th", name="th")
                    nc.scalar.activation(th[:], ct[:], AFT.Tanh)
                    go = gate(2)
                    ht = opool.tile([128, w], F32, tag="ht", name="ht")
                    nc.vector.tensor_tensor(ht[:], go[:], th[:], ALU.mult)

                    osl = slice(bb * BBLK + s * w, bb * BBLK + (s + 1) * w)
                    out_ring.dma_start(out=cTo_r[:, j, osl], in_=ct[:])
                    out_ring.dma_start(out=hT_r[:, j, osl], in_=ht[:])

            # ---- batch block 0: k-outer over both j's, chasing the DMA
            # stream (8 MMs per arriving k-chunk matches the ~2.2us/chunk
            # delivery rate) ----
            cprev0 = load_cprev(0)
            ps0 = [
                [pp.tile([128, BBLK], F32, tag=f"ps{g}", name=f"ps{g}") for g in range(4)]
                for j in range(NJ)
            ]
            for k in range(KT):
                for j in range(NJ):
                    for g in (3, 0, 1, 2):  # c-gate first: longest elementwise chain
                        nc.tensor.matmul(
                            ps0[j][g][:],
                            w_sb[k][:, g * U + j * 128: g * U + (j + 1) * 128],
                            x0_sb[k][:],
                            start=(k == 0),
                            stop=(k == KT - 1),
                        )
            for j in range(NJ):
                elementwise(ps0[j], cprev0, 0, j)

            # ---- batch blocks 1..NBB-1: gate-outer, k-inner ----
            for bb in range(1, NBB):
                bsl = slice(bb * BBLK, (bb + 1) * BBLK)
                x_sb = []
                for kp in range(KT // 2):
                    xt = xpool.tile(
                        [128, 2, BBLK], F32R, tag=f"xp{kp}", name=f"xp{kp}", bufs=3
                    )
                    nc.sync.dma_start(out=xt[:], in_=xT_r[:, 2 * kp:2 * kp + 2, bsl])
                    x_sb.extend([xt[:, 0, :], xt[:, 1, :]])
                cprev_sb = load_cprev(bb)

                for j in range(NJ):
                    ps = [None] * 4
                    for g in (3, 0, 1, 2):  # c-gate first: longest elementwise chain
                        pst = pp.tile([128, BBLK], F32, tag=f"ps{g}", name=f"ps{g}")
                        col0 = g * U + j * 128
                        for k in range(KT):
                            nc.tensor.matmul(
                                pst[:],
                                w_sb[k][:, col0:col0 + 128],
                                x_sb[k][:],
                                start=(k == 0),
                                stop=(k == KT - 1),
                            )
                        ps[g] = pst
                    last = (bb == NBB - 1) and (j == NJ - 1)
                    elementwise(
                        ps, cprev_sb, bb, j,
                        n_split=2 if last else 1,
                        out_ring=nc.scalar if last else None,
                    )

    nc.finalize()
    return nc


def _shard_inputs(e_t, h_prev, c_prev, W_x, b_x, W_h, b_h, b_extra):
    e_t = np.ascontiguousarray(np.asarray(e_t, dtype=np.float32))
    h_prev = np.ascontiguousarray(np.asarray(h_prev, dtype=np.float32))
    c_prev = np.ascontiguousarray(np.asarray(c_prev, dtype=np.float32))
    W_x = np.asarray(W_x, dtype=np.float32)
    W_h = np.asarray(W_h, dtype=np.float32)
    bias_full = (
        np.asarray(b_x, dtype=np.float32)
        + np.asarray(b_h, dtype=np.float32)
        + np.asarray(b_extra, dtype=np.float32)
    )

    # X^T = [e_t | h_prev]^T : [K, B]
    XT = np.empty((K, B), dtype=np.float32)
    XT[:D] = e_t.T
    XT[D:] = h_prev.T
    W = np.concatenate([W_x, W_h], axis=1)  # [4H, K]

    in_maps = []
    for core in range(M_BATCH * M_UNIT):
        m, q = divmod(core, M_UNIT)
        rows = np.concatenate(
            [np.arange(g0 + q * U, g0 + (q + 1) * U) for g0 in (0, H, 2 * H, 3 * H)]
        )
        in_maps.append({
            "xT": np.ascontiguousarray(XT[:, m * BS:(m + 1) * BS]),
            "wT": np.ascontiguousarray(W[rows].T),
            "bias": np.ascontiguousarray(bias_full[rows]),
            "cT": np.ascontiguousarray(c_prev[m * BS:(m + 1) * BS, q * U:(q + 1) * U].T),
        })
    return in_maps


def _assemble_outputs(results):
    h_t = np.empty((B, H), dtype=np.float32)
    c_t = np.empty((B, H), dtype=np.float32)
    for core, res in enumerate(results):
        m, q = divmod(core, M_UNIT)
        h_t[m * BS:(m + 1) * BS, q * U:(q + 1) * U] = res["hT_out"].T
        c_t[m * BS:(m + 1) * BS, q * U:(q + 1) * U] = res["cT_out"].T
    return h_t, c_t


def kernel(e_t, h_prev, c_prev, W_x, b_x, W_h, b_h, b_extra, _runner=None):
    in_maps = _shard_inputs(e_t, h_prev, c_prev, W_x, b_x, W_h, b_h, b_extra)
    nc = _build_nc()
    if _runner is None:
        res = run_bass_kernel_spmd(nc, in_maps, core_ids=list(range(8)))
        results = res.results
    else:
        results = _runner(nc, in_maps)
    return _assemble_outputs(results)


# revision 24
# speedup vs baseline: 1.0438x; 1.0438x over previous
"""CustomLSTMCell fused kernel for 8x Trainium2 NeuronCores.

Reference computation (B=8192, D=H=1024):
    z = e_t @ W_x.T + h_prev @ W_h.T + (b_x + b_h + b_extra)   # [B, 4H]
    f, i, o, c = split(z, 4)
    c_t = sigmoid(f) * c_prev + sigmoid(i) * tanh(c)
    h_t = sigmoid(o) * tanh(c_t)

Sharding: 2-way batch x 4-way hidden-unit (8 cores, no collectives).
Each core computes z transposed ([gate_rows, batch] layout) so the bias
folds into the ScalarE activation's per-partition bias operand, and both
matmul operands arrive pre-transposed from the host (contraction dim on
partitions).  Matmuls run in float32r (full-rate fp32 path on TRN2 for
moving free-dim >= 256, ~2^-13 effective mantissa).

Schedule notes (measured on HW, ~267us total; PE matmul floor is ~240us):
 - W and the first batch-block's x are DMAed as interleaved per-k-chunk
   tiles on the sync HWDGE ring; the first batch block runs its matmuls
   k-outer so the PE starts as soon as chunk 0 lands (~13us in) and
   chases the arrival stream instead of waiting ~43us for the full 12MB.
 - Each DMA_DIRECT2D costs ~650ns of issue time on its engine, and a DMA
   chain on the scalar engine develops issue-blocking sem waits that
   delay ACTIVATEs (which gate PSUM-bank release).  So: bulk input
   stream on sync, cprev + steady-state outputs on gpsimd (SWDGE), and
   the scalar ring carries only the bias load and the final outputs.
 - The final quadruple's elementwise runs in two half-width chunks to
   shorten the post-matmul tail; the Tile epilogue (drain + sem-reset
   butterfly) is ~10us of fixed cost after that.
"""

import sys

if "/opt/trn_rl_repo" not in sys.path:
    sys.path.insert(0, "/opt/trn_rl_repo")

import numpy as np

import concourse.bass as bass
import concourse.mybir as mybir
from concourse import bacc
from concourse.bass_utils import run_bass_kernel_spmd
from concourse.tile import TileContext

F32 = mybir.dt.float32
F32R = mybir.dt.float32r
AFT = mybir.ActivationFunctionType
ALU = mybir.AluOpType

B, D, H = 8192, 1024, 1024
M_BATCH, M_UNIT = 2, 4          # batch split x unit split = 8 cores
BS = B // M_BATCH               # 4096 batch rows per core
U = H // M_UNIT                 # 256 hidden units per core
K = D + H                       # 2048 contraction (e_t | h_prev)
KT = K // 128                   # 16 k-chunks
G = 4 * U                       # 1024 gate rows per core (f|i|o|c x U)
BBLK = 512                      # moving free-dim per matmul
NBB = BS // BBLK                # 8 batch blocks
NJ = U // 128                   # 2 unit sub-blocks of 128 partitions

GATE_FUNCS = [AFT.Sigmoid, AFT.Sigmoid, AFT.Sigmoid, AFT.Tanh]  # f, i, o, c


def _build_nc():
    nc = bacc.Bacc()

    xT = nc.dram_tensor("xT", [K, BS], F32R, kind="ExternalInput")
    wT = nc.dram_tensor("wT", [K, G], F32R, kind="ExternalInput")
    bias = nc.dram_tensor("bias", [G], F32, kind="ExternalInput")
    cT = nc.dram_tensor("cT", [U, BS], F32, kind="ExternalInput")
    hT_out = nc.dram_tensor("hT_out", [U, BS], F32, kind="ExternalOutput")
    cT_out = nc.dram_tensor("cT_out", [U, BS], F32, kind="ExternalOutput")

    xT_r = xT.ap().rearrange("(k p) b -> p k b", p=128)      # [128, KT, BS]
    wT_r = wT.ap().rearrange("(k p) g -> p k g", p=128)      # [128, KT, G]
    bias_r = bias.ap().rearrange("(c p) -> p c", p=128)      # [128, 4*NJ]
    cT_r = cT.ap().rearrange("(j p) b -> p j b", p=128)      # [128, NJ, BS]
    hT_r = hT_out.ap().rearrange("(j p) b -> p j b", p=128)
    cTo_r = cT_out.ap().rearrange("(j p) b -> p j b", p=128)

    with TileContext(nc) as tc:
        with (
            tc.tile_pool(name="wpool", bufs=1) as wpool,
            tc.tile_pool(name="xpool", bufs=2) as xpool,
            tc.tile_pool(name="cpool", bufs=2) as cpool,
            tc.tile_pool(name="gpool", bufs=2) as gpool,
            tc.tile_pool(name="opool", bufs=2) as opool,
            tc.tile_pool(name="psum", bufs=2, space="PSUM") as pp,
        ):
            # Per-k W tiles, DMA-interleaved with the first batch block's x
            # tiles so the PE can start on chunk 0 immediately and chase the
            # arrival stream.  All on the sync ring: its ~650ns/DMA issue
            # rate paces arrivals in k-order, and the scalar engine must stay
            # free for ACTIVATEs (a DMA chain there develops issue-blocking
            # sem waits that stall PSUM release by ~8us).
            bias_sb = wpool.tile([128, 4 * NJ], F32)
            nc.scalar.dma_start(out=bias_sb[:], in_=bias_r)

            # PE warm-up: ~12 throwaway matmuls on zeroed scratch while the
            # first W/x chunks are still in flight.  The HAM clock gate needs
            # ~3.4us of sustained PE activity to lift the PE from 1.2GHz to
            # 2.4GHz; without this the first ~13 real matmuls run cold.
            warm = wpool.tile([128, BBLK], F32, name="warm")
            nc.vector.memset(warm[:], 0.0)
            warm_r = warm.bitcast(F32R)
            warm_ps = pp.tile([128, BBLK], F32, tag="ps0", name="warm_ps")
            # 12 cold-rate matmuls ~= 5us: enough sustained activity to trip
            # the HAM SHORT window, ending about when the first W/x chunks
            # land so real matmuls start warm.
            for _ in range(12):
                nc.tensor.matmul(
                    warm_ps[:], warm_r[:, 0:128], warm_r[:], start=True, stop=True
                )

            w_sb = []
            x0_sb = []
            for k in range(KT):
                wt = wpool.tile([128, G], F32R, tag=f"w{k}", name=f"w{k}")
                nc.sync.dma_start(out=wt[:], in_=wT_r[:, k, :])
                w_sb.append(wt)
                if k % 2 == 0:
                    xp = xpool.tile(
                        [128, 2, BBLK], F32R, tag=f"xp{k // 2}", name=f"xp{k // 2}", bufs=3
                    )
                nc.sync.dma_start(out=xp[:, k % 2, :], in_=xT_r[:, k, 0:BBLK])
                x0_sb.append(xp[:, k % 2, :])

            def load_cprev(bb):
                t = cpool.tile([128, NJ, BBLK], F32, tag="cprev", name="cprev")
                nc.gpsimd.dma_start(
                    out=t[:], in_=cT_r[:, :, bb * BBLK:(bb + 1) * BBLK]
                )
                return t

            def elementwise(ps, cprev_sb, bb, j, n_split=1, out_ring=None):
                """Gate nonlinearities + cell update for one quadruple."""
                out_ring = out_ring or nc.gpsimd
                w = BBLK // n_split
                for s in range(n_split):
                    csl = slice(s * w, (s + 1) * w)

                    def gate(g):
                        at = gpool.tile([128, w], F32, tag=f"act{g}", name=f"act{g}")
                        nc.scalar.activation(
                            at[:], ps[g][:, csl], GATE_FUNCS[g],
                            bias=bias_sb[:, 2 * g + j: 2 * g + j + 1],
                        )
                        return at

                    # ACT stream order mirrors the (c,f,i,o) matmul order so
                    # the c_t chain completes before the o-gate's matmuls do.
                    mc = gate(3)
                    gf = gate(0)
                    gi = gate(1)
                    t1 = gpool.tile([128, w], F32, tag="t1", name="t1")
                    nc.vector.tensor_tensor(
                        t1[:], gf[:], cprev_sb[:, j, csl], ALU.mult
                    )
                    t2 = gpool.tile([128, w], F32, tag="t2", name="t2")
                    nc.vector.tensor_tensor(t2[:], gi[:], mc[:], ALU.mult)
                    ct = opool.tile([128, w], F32, tag="ct", name="ct")
                    nc.vector.tensor_tensor(ct[:], t1[:], t2[:], ALU.add)
                    th = gpool.tile([128, w], F32, tag="th", name="th")
                    nc.scalar.activation(th[:], ct[:], AFT.Tanh)
                    go = gate(2)
                    ht = opool.tile([128, w], F32, tag="ht", name="ht")
                    nc.vector.tensor_tensor(ht[:], go[:], th[:], ALU.mult)

                    osl = slice(bb * BBLK + s * w, bb * BBLK + (s + 1) * w)
                    out_ring.dma_start(out=cTo_r[:, j, osl], in_=ct[:])
                    out_ring.dma_start(out=hT_r[:, j, osl], in_=ht[:])

            # ---- batch block 0: k-outer over both j's, chasing the DMA
            # stream (8 MMs per arriving k-chunk matches the ~2.2us/chunk
            # delivery rate) ----
            cprev0 = load_cprev(0)
            ps0 = [
                [pp.tile([128, BBLK], F32, tag=f"ps{g}", name=f"ps{g}") for g in range(4)]
                for j in range(NJ)
            ]
            for k in range(KT):
                for j in range(NJ):
                    for g in (3, 0, 1, 2):  # c-gate first: longest elementwise chain
                        nc.tensor.matmul(
                            ps0[j][g][:],
                            w_sb[k][:, g * U + j * 128: g * U + (j + 1) * 128],
                            x0_sb[k][:],
                            start=(k == 0),
                            stop=(k == KT - 1),
                        )
            for j in range(NJ):
                elementwise(ps0[j], cprev0, 0, j)

            # ---- batch blocks 1..NBB-1: gate-outer, k-inner ----
            for bb in range(1, NBB):
                bsl = slice(bb * BBLK, (bb + 1) * BBLK)
                x_sb = []
                for kp in range(KT // 2):
                    xt = xpool.tile(
                        [128, 2, BBLK], F32R, tag=f"xp{kp}", name=f"xp{kp}", bufs=3
                    )
                    nc.sync.dma_start(out=xt[:], in_=xT_r[:, 2 * kp:2 * kp + 2, bsl])
                    x_sb.extend([xt[:, 0, :], xt[:, 1, :]])
                cprev_sb = load_cprev(bb)

                for j in range(NJ):
                    ps = [None] * 4
                    for g in (3, 0, 1, 2):  # c-gate first: longest elementwise chain
                        pst = pp.tile([128, BBLK], F32, tag=f"ps{g}", name=f"ps{g}")
                        col0 = g * U + j * 128
                        for k in range(KT):
                            nc.tensor.matmul(
                                pst[:],
                                w_sb[k][:, col0:col0 + 128],
                                x_sb[k][:],
                                start=(k == 0),
                                stop=(k == KT - 1),
                            )
                        ps[g] = pst
                    last = (bb == NBB - 1) and (j == NJ - 1)
                    elementwise(
                        ps, cprev_sb, bb, j,
                        n_split=2 if last else 1,
                        out_ring=nc.scalar if last else None,
                    )

    nc.finalize()
    return nc


def _shard_inputs(e_t, h_prev, c_prev, W_x, b_x, W_h, b_h, b_extra):
    e_t = np.ascontiguousarray(np.asarray(e_t, dtype=np.float32))
    h_prev = np.ascontiguousarray(np.asarray(h_prev, dtype=np.float32))
    c_prev = np.ascontiguousarray(np.asarray(c_prev, dtype=np.float32))
    W_x = np.asarray(W_x, dtype=np.float32)
    W_h = np.asarray(W_h, dtype=np.float32)
    bias_full = (
        np.asarray(b_x, dtype=np.float32)
        + np.asarray(b_h, dtype=np.float32)
        + np.asarray(b_extra, dtype=np.float32)
    )

    # X^T = [e_t | h_prev]^T : [K, B]
    XT = np.empty((K, B), dtype=np.float32)
    XT[:D] = e_t.T
    XT[D:] = h_prev.T
    W = np.concatenate([W_x, W_h], axis=1)  # [4H, K]

    in_maps = []
    for core in range(M_BATCH * M_UNIT):
        m, q = divmod(core, M_UNIT)
        rows = np.concatenate(
            [np.arange(g0 + q * U, g0 + (q + 1) * U) for g0 in (0, H, 2 * H, 3 * H)]
        )
        in_maps.append({
            "xT": np.ascontiguousarray(XT[:, m * BS:(m + 1) * BS]),
            "wT": np.ascontiguousarray(W[rows].T),
            "bias": np.ascontiguousarray(bias_full[rows]),
            "cT": np.ascontiguousarray(c_prev[m * BS:(m + 1) * BS, q * U:(q + 1) * U].T),
        })
    return in_maps


def _assemble_outputs(results):
    h_t = np.empty((B, H), dtype=np.float32)
    c_t = np.empty((B, H), dtype=np.float32)
    for core, res in enumerate(results):
        m, q = divmod(core, M_UNIT)
        h_t[m * BS:(m + 1) * BS, q * U:(q + 1) * U] = res["hT_out"].T
        c_t[m * BS:(m + 1) * BS, q * U:(q + 1) * U] = res["cT_out"].T
    return h_t, c_t


def kernel(e_t, h_prev, c_prev, W_x, b_x, W_h, b_h, b_extra, _runner=None):
    in_maps = _shard_inputs(e_t, h_prev, c_prev, W_x, b_x, W_h, b_h, b_extra)
    nc = _build_nc()
    if _runner is None:
        res = run_bass_kernel_spmd(nc, in_maps, core_ids=list(range(8)))
        results = res.results
    else:
        results = _runner(nc, in_maps)
    return _assemble_outputs(results)
